# revision 1
# baseline (speedup 1.0000x reference)
"""Self-contained Trainium2 Bass kernel for BoSs (block-of-states) attention.

Strategy (8 NeuronCores):
  - data-parallel over batch (2) x tensor-parallel over heads (4):
    core c handles batch c//4, q-heads [4g:4g+4] and kv-head g where g=c%4.
  - host packs tokens by state id (stable sort) so the BoSs mask becomes
    block-banded causal in packed coordinates (max segment ~280 << WIN=1024,
    so the sliding window never binds and a 3-block lookback band suffices).
  - transposed activation layouts ([dim, seq]) keep every matmul contracting
    over the partition axis; scores are computed transposed ([k, q]) so the
    attention-weights matrix feeds the AV matmul without transposes.
  - fp16 matmul operands (full PE rate + FWL), fp32 PSUM accumulation.
  - additive {0,-30000} mask applied to scores in fp32 PSUM before exp, so
    no max-subtraction is needed (scores are bounded) and masked scores
    never overflow the fp16 attention-weight tiles.
  - softmax denominator via a ones-vector matmul; 1/l broadcast across
    partitions via a K=1 matmul; partial row-parallel Wo outputs are summed
    and unpermuted on host.
"""

import numpy as np
from contextlib import ExitStack

# problem constants (hardcoded per spec)
B, L, HID = 2, 2048, 2048
H, KVH, D = 16, 4, 128
THETA = 10000.0
NCORES = 8
TP = 4            # tensor-parallel group size (cores per batch)
QH = H // TP      # q heads per core = 4
QCH = 256         # q columns per attention chunk
NJQ = L // QCH    # 8
NKB = L // 128    # 16 k-blocks / q-blocks
NHC = HID // 128  # 16 hidden-dim chunks
LC = 512          # phase-1 L-chunk width
NLC = L // LC     # 4
BAND_BACK = 3     # k-block lookback; correct while max segment <= 385
SCALE = float(D) ** -0.5
# uniform logit shift (in raw-score units) folded into the additive mask;
# keeps exp() outputs comfortably inside fp16 range and cancels exactly in
# the softmax normalization (same constant for every valid entry).
MASK_SHIFT = -2.0 / SCALE
MASK_NEG = -30000.0


def _band(jq):
    lo = max(0, 2 * jq - BAND_BACK)
    hi = 2 * jq + 1
    return lo, hi


_BANDS = [_band(j) for j in range(NJQ)]
_NBLK = sum(hi - lo + 1 for lo, hi in _BANDS)
_MBASE = np.cumsum([0] + [hi - lo + 1 for lo, hi in _BANDS]).tolist()

_CACHE = {}
LAST_EXEC_NS = None
LAST_RUN_WALL_S = None


def _build_nc():
    import concourse.tile as tile
    from concourse import bacc, mybir

    f32 = mybir.dt.float32
    f16 = mybir.dt.float16
    EXP = mybir.ActivationFunctionType.Exp

    nc = bacc.Bacc(
        "TRN2", target_bir_lowering=False, debug=False, num_devices=NCORES
    )

    xT = nc.dram_tensor("xT", [HID, L], f16, kind="ExternalInput").ap()
    wq = nc.dram_tensor("wq", [HID, QH * D], f16, kind="ExternalInput").ap()
    wk = nc.dram_tensor("wk", [HID, D], f16, kind="ExternalInput").ap()
    wv = nc.dram_tensor("wv", [HID, D], f16, kind="ExternalInput").ap()
    wo = nc.dram_tensor("wo", [QH * D, HID], f16, kind="ExternalInput").ap()
    cosd = nc.dram_tensor("cosd", [D, L], f16, kind="ExternalInput").ap()
    sind = nc.dram_tensor("sind", [D, L], f16, kind="ExternalInput").ap()
    mskd = nc.dram_tensor("mskd", [_NBLK, 128, QCH], f16, kind="ExternalInput").ap()
    swpd = nc.dram_tensor("swpd", [128, 128], f16, kind="ExternalInput").ap()
    idnd = nc.dram_tensor("idnd", [128, 128], f16, kind="ExternalInput").ap()
    out = nc.dram_tensor("out", [L, HID], f16, kind="ExternalOutput").ap()

    with tile.TileContext(nc) as tc, ExitStack() as top:
        persist = top.enter_context(tc.tile_pool(name="persist", bufs=1))
        kT = persist.tile([128, L], f16, tag="kT", name="kT")
        qT = [
            persist.tile([128, L], f16, tag=f"qT{h}", name=f"qT{h}")
            for h in range(QH)
        ]
        oT = [
            persist.tile([128, L], f16, tag=f"oT{h}", name=f"oT{h}")
            for h in range(QH)
        ]
        vA = persist.tile([128, NKB, 128], f16, tag="vA", name="vA")
        cosT = persist.tile([128, L], f16, tag="cosT", name="cosT")
        sinT = persist.tile([128, L], f16, tag="sinT", name="sinT")
        ones = persist.tile([128, 128], f16, tag="ones", name="ones")
        swp = persist.tile([128, 128], f16, tag="swp", name="swp")
        idn = persist.tile([128, 128], f16, tag="idn", name="idn")

        nc.any.memset(ones[:], 1.0)

        # weights / inputs (live whole kernel; everything coexists so the
        # scheduler can overlap phases by data deps alone)
        wpool = top.enter_context(tc.tile_pool(name="wpool", bufs=1))
        wq_s = wpool.tile([128, NHC, QH * D], f16, tag="wq", name="wq_s")
        wk_s = wpool.tile([128, NHC, D], f16, tag="wk", name="wk_s")
        wv_s = wpool.tile([128, NHC, D], f16, tag="wv", name="wv_s")
        vT_s = wpool.tile([128, L], f16, tag="vT", name="vT_s")
        wo_s = wpool.tile([128, QH, HID], f16, tag="wo", name="wo_s")
        xpool = top.enter_context(tc.tile_pool(name="xpool", bufs=2))

        # DMA emission order = first-needed-first: k/v weights + x chunk 0
        # interleaved, then q weights, then rope tables; wo after phase 1.
        # Early DMAs are spread over four sequencers: single-queue issue is
        # ~650ns per dma_start, which would serialize the startup stream.
        qs_engines = [nc.sync, nc.scalar, nc.gpsimd]
        xt0 = xpool.tile([128, NHC, LC], f16, tag="x", name="xt0")
        for c in range(NHC):
            eng = qs_engines[c % 3]
            eng.dma_start(wk_s[:, c, :], wk[c * 128 : (c + 1) * 128, :])
            eng.dma_start(xt0[:, c, :], xT[c * 128 : (c + 1) * 128, 0:LC])
            eng.dma_start(wv_s[:, c, :], wv[c * 128 : (c + 1) * 128, :])
            eng.dma_start(wq_s[:, c, :], wq[c * 128 : (c + 1) * 128, :])
        nc.scalar.dma_start(swp[:], swpd[:])
        nc.sync.dma_start(cosT[:], cosd[:])
        nc.sync.dma_start(sinT[:], sind[:])
        nc.gpsimd.dma_start(idn[:], idnd[:])
        tpool = top.enter_context(tc.tile_pool(name="tpool", bufs=3))
        mpool = top.enter_context(tc.tile_pool(name="mpool", bufs=2))
        ppool = top.enter_context(tc.tile_pool(name="ppool", bufs=2))
        spool = top.enter_context(tc.tile_pool(name="spool", bufs=2))
        # PSUM: 8 banks total. big([128,512] f32 = 1 bank) x3 for
        # projections/swap/vtr/final; S([128,3,256] f32 = 2 banks) x2;
        # o(1 bank) x1; l(1 bank) x1.
        psB = top.enter_context(tc.tile_pool(name="psB", bufs=2, space="PSUM"))
        psS = top.enter_context(tc.tile_pool(name="psS", bufs=2, space="PSUM"))
        psO = top.enter_context(tc.tile_pool(name="psO", bufs=1, space="PSUM"))
        psL = top.enter_context(tc.tile_pool(name="psL", bufs=1, space="PSUM"))

        # ---- phase 1: projections (qT/kT rope'd, v transposed) ----
        for lc in range(NLC):
            cols = slice(lc * LC, (lc + 1) * LC)
            if lc == 0:
                xt = xt0
            else:
                xt = xpool.tile([128, NHC, LC], f16, tag="x", name=f"xt{lc}")
                for c in range(NHC):
                    eng = nc.sync if c % 2 == 0 else nc.gpsimd
                    eng.dma_start(
                        xt[:, c, :], xT[c * 128 : (c + 1) * 128, cols]
                    )
            # k first: its (small) weights arrive earliest, so PE starts sooner
            for hb in (QH, QH + 1, 0, 1, 2, 3):
                ps = psB.tile([128, LC], f32, tag="big", name=f"ps{lc}_{hb}")
                for c in range(NHC):
                    if hb < QH:
                        lhsT = wq_s[:, c, hb * 128 : (hb + 1) * 128]
                    elif hb == QH:
                        lhsT = wk_s[:, c, :]
                    else:
                        lhsT = wv_s[:, c, :]
                    nc.tensor.matmul(
                        ps[:],
                        lhsT,
                        xt[:, c, :],
                        start=(c == 0),
                        stop=(c == NHC - 1),
                    )
                if hb <= QH:  # rope for q & k
                    dst = qT[hb] if hb < QH else kT
                    plain = tpool.tile(
                        [128, LC], f16, tag="plain", name=f"pl{lc}_{hb}"
                    )
                    nc.scalar.copy(plain[:], ps[:])
                    sw = psB.tile([128, LC], f32, tag="big", name=f"sw{lc}_{hb}")
                    nc.tensor.matmul(sw[:], swp[:], plain[:], start=True, stop=True)
                    t1 = tpool.tile([128, LC], f16, tag="t1", name=f"t1_{lc}_{hb}")
                    nc.gpsimd.tensor_mul(t1[:], plain[:], cosT[:, cols])
                    t2 = tpool.tile([128, LC], f16, tag="t2", name=f"t2_{lc}_{hb}")
                    nc.vector.tensor_mul(t2[:], sw[:], sinT[:, cols])
                    nc.gpsimd.tensor_add(dst[:, cols], t1[:], t2[:])
                else:  # v: keep transposed copy, then transpose this chunk
                    nc.scalar.copy(vT_s[:, cols], ps[:])
                    for kb in range(lc * (LC // 128), (lc + 1) * (LC // 128)):
                        vt_ps = psB.tile(
                            [128, 128], f32, tag="big", name=f"vt{kb}"
                        )
                        nc.tensor.matmul(
                            vt_ps[:],
                            vT_s[:, kb * 128 : (kb + 1) * 128],
                            idn[:],
                            start=True,
                            stop=True,
                        )
                        nc.scalar.copy(vA[:, kb, :], vt_ps[:])

        nc.sync.dma_start(wo_s[:], wo.rearrange("(h p) n -> p h n", p=128))

        # ---- phase 2: banded attention in transposed layout ----
        SB = 3  # S sub-chunk width in k-blocks (2 PSUM banks)
        for jq in range(NJQ):
            lo, hi = _BANDS[jq]
            nkb = hi - lo + 1
            qs = slice(jq * QCH, (jq + 1) * QCH)
            msk = mpool.tile([128, nkb, QCH], f16, tag="m", name=f"msk{jq}")
            nc.sync.dma_start(
                msk[:],
                mskd[_MBASE[jq] : _MBASE[jq] + nkb].rearrange("k p n -> p k n"),
            )
            for h in range(QH):
                P = ppool.tile([128, nkb, QCH], f16, tag="P", name=f"p{jq}_{h}")
                for p0 in range(0, nkb, SB):
                    pn = min(SB, nkb - p0)
                    s_ps = psS.tile(
                        [128, SB, QCH], f32, tag="S", name=f"s{jq}_{h}_{p0}"
                    )
                    for i in range(p0, p0 + pn):
                        kb = lo + i
                        nc.tensor.matmul(
                            s_ps[:, i - p0, :],
                            kT[:, kb * 128 : (kb + 1) * 128],
                            qT[h][:, qs],
                            start=True,
                            stop=True,
                        )
                    # additive mask in fp32 PSUM (in-place), then exp -> fp16
                    nc.vector.tensor_add(
                        s_ps[:, :pn, :], s_ps[:, :pn, :], msk[:, p0 : p0 + pn, :]
                    )
                    nc.scalar.activation(
                        P[:, p0 : p0 + pn, :], s_ps[:, :pn, :], EXP, scale=SCALE
                    )
                l_ps = psL.tile([1, QCH], f32, tag="l", name=f"l{jq}_{h}")
                for i in range(nkb):
                    nc.tensor.matmul(
                        l_ps[:],
                        ones[:, 0:1],
                        P[:, i, :],
                        start=(i == 0),
                        stop=(i == nkb - 1),
                    )
                o_ps = psO.tile([128, QCH], f32, tag="o", name=f"o{jq}_{h}")
                for i in range(nkb):
                    kb = lo + i
                    nc.tensor.matmul(
                        o_ps[:],
                        vA[:, kb, :],
                        P[:, i, :],
                        start=(i == 0),
                        stop=(i == nkb - 1),
                    )
                rc = spool.tile([1, QCH], f16, tag="lsb", name=f"ls{jq}_{h}")
                with nc.allow_low_precision(
                    reason="fp16 1/l scales fp16 outputs; 5e-4 rel ok"
                ):
                    nc.vector.reciprocal(rc[:], l_ps[:])
                r_bc = spool.tile([128, QCH], f16, tag="lbc", name=f"lb{jq}_{h}")
                nc.gpsimd.partition_broadcast(r_bc[:], rc[:])
                nc.vector.tensor_mul(oT[h][:, qs], o_ps[:], r_bc[:])

        # ---- phase 3: output projection (row-parallel partial) ----
        for qb in range(NKB):
            for hc in range(HID // 512):
                # late groups borrow the attention pool's idle banks so the
                # PSUM->SBUF copy isn't on the matmul critical path
                if qb >= 10 and (qb * 4 + hc) % 2 == 0:
                    f_ps = psS.tile(
                        [128, 512], f32, tag="S", name=f"f{qb}_{hc}"
                    )
                else:
                    f_ps = psB.tile(
                        [128, 512], f32, tag="big", name=f"f{qb}_{hc}"
                    )
                for h in range(QH):
                    nc.tensor.matmul(
                        f_ps[:],
                        oT[h][:, qb * 128 : (qb + 1) * 128],
                        wo_s[:, h, hc * 512 : (hc + 1) * 512],
                        start=(h == 0),
                        stop=(h == QH - 1),
                    )
                ob = spool.tile(
                    [128, 512], f16, tag="ob", bufs=4, name=f"ob{qb}_{hc}"
                )
                nc.any.tensor_copy(ob[:], f_ps[:])
                nc.sync.dma_start(
                    out[qb * 128 : (qb + 1) * 128, hc * 512 : (hc + 1) * 512],
                    ob[:],
                )

    nc.compile()
    return nc


def _get_nc():
    if "nc" not in _CACHE:
        _CACHE["nc"] = _build_nc()
    return _CACHE["nc"]


def kernel(hidden_states, Wq, Wk, Wv, Wo, sid, position_ids):
    global LAST_EXEC_NS, LAST_RUN_WALL_S
    import time

    from concourse.bass_utils import run_bass_kernel_spmd

    hidden = np.asarray(hidden_states, dtype=np.float32)
    Wq = np.asarray(Wq, dtype=np.float32)
    Wk = np.asarray(Wk, dtype=np.float32)
    Wv = np.asarray(Wv, dtype=np.float32)
    Wo = np.asarray(Wo, dtype=np.float32)
    sid = np.asarray(sid)
    position_ids = np.asarray(position_ids)

    nc = _get_nc()

    f16 = np.float16
    swp = np.zeros((128, 128), f16)
    swp[(np.arange(128) + 64) % 128, np.arange(128)] = 1.0
    idn = np.eye(128, dtype=f16)

    in_maps = []
    perms = []
    for b in range(B):
        s = sid[b].astype(np.int64)
        perm = np.argsort(s, kind="stable")
        perms.append(perm)
        st = s[perm]
        seg_max = int(np.bincount(st, minlength=1).max())
        assert seg_max <= BAND_BACK * 128 + 1, (
            f"segment length {seg_max} exceeds supported band"
        )

        pos = position_ids[b][perm].astype(np.float32)
        inv = (
            1.0
            / (THETA ** (np.arange(0, D, 2, dtype=np.float32) / np.float32(D)))
        ).astype(np.float32)
        fr = pos[:, None] * inv[None, :]
        emb = np.concatenate([fr, fr], axis=1)  # [L, D]
        cosT = np.ascontiguousarray(np.cos(emb).T.astype(f16))
        sinT = np.sin(emb).T.astype(np.float32).copy()
        sinT[: D // 2] *= -1.0  # fold rotate_half sign
        sinT = np.ascontiguousarray(sinT.astype(f16))

        xTp = np.ascontiguousarray(hidden[b].T[:, perm].astype(f16))

        msk = np.full((_NBLK, 128, QCH), MASK_NEG, f16)
        ki = np.arange(128)
        qi = np.arange(QCH)
        for jq in range(NJQ):
            lo, hi = _BANDS[jq]
            for i in range(hi - lo + 1):
                kb = lo + i
                kidx = kb * 128 + ki
                qidx = jq * QCH + qi
                m = (st[kidx][:, None] == st[qidx][None, :]) & (
                    kidx[:, None] <= qidx[None, :]
                )
                msk[_MBASE[jq] + i] = np.where(m, MASK_SHIFT, MASK_NEG).astype(f16)

        for g in range(TP):
            in_maps.append(
                dict(
                    xT=xTp,
                    wq=np.ascontiguousarray(Wq[g * 512 : (g + 1) * 512].T.astype(f16)),
                    wk=np.ascontiguousarray(Wk[g * 128 : (g + 1) * 128].T.astype(f16)),
                    wv=np.ascontiguousarray(Wv[g * 128 : (g + 1) * 128].T.astype(f16)),
                    wo=np.ascontiguousarray(
                        Wo[:, g * 512 : (g + 1) * 512].T.astype(f16)
                    ),
                    cosd=cosT,
                    sind=sinT,
                    mskd=msk,
                    swpd=swp,
                    idnd=idn,
                )
            )

    t0 = time.time()
    res = run_bass_kernel_spmd(nc, in_maps, core_ids=list(range(NCORES)))
    LAST_RUN_WALL_S = time.time() - t0
    LAST_EXEC_NS = res.exec_time_ns

    full = np.empty((B, L, HID), np.float32)
    for b in range(B):
        acc = np.asarray(res.results[4 * b]["out"]).astype(np.float32)
        for g in range(1, TP):
            acc += np.asarray(res.results[4 * b + g]["out"]).astype(np.float32)
        unp = np.empty_like(acc)
        unp[perms[b]] = acc
        full[b] = unp
    return full



# revision 19
# speedup vs baseline: 1.0426x; 1.0426x over previous
"""Self-contained Trainium2 Bass kernel for BoSs (block-of-states) attention.

Strategy (8 NeuronCores):
  - data-parallel over batch (2) x tensor-parallel over heads (4):
    core c handles batch c//4, q-heads [4g:4g+4] and kv-head g where g=c%4.
  - host packs tokens by state id (stable sort) so the BoSs mask becomes
    block-banded causal in packed coordinates (max segment <=385 with
    BAND_BACK=3, so the WIN=1024 sliding window never binds).
  - Q/K/V and O projections run as 3-term hi/lo fp8 DoubleRow matmuls
    (y = xh@Wh + xl@Wh + xh@Wl at 0.75x the fp16 PE cost but ~fp16
    accuracy). Weights are pre-scaled by 32 so the fp8 lo-residuals stay
    out of e4m3's denormal range; the scale folds exactly into the exp
    scale, a 32-valued ones-vector for the softmax denominator, and a
    final /32 on the host.
  - scores/AV/denominator stay fp16; scores are computed transposed
    ([k, q]) so attention weights feed the AV matmul without transposes.
  - exp runs on Act with a uniform -3 bias (cancels in softmax, keeps
    1/l inside fp16 normal range); the BoSs mask is applied
    MULTIPLICATIVELY ({0,1} fp16) after exp on DVE's 4x mode.
  - V projection is computed directly transposed (vA[k,d] = x^T Wv^T).
  - phase interleaving: projection chunk lc -> attention jq=2lc,2lc+1 ->
    output projection qb=4lc..4lc+3, so PE never waits on a whole phase.
"""

import numpy as np
from contextlib import ExitStack

# problem constants (hardcoded per spec)
B, L, HID = 2, 2048, 2048
H, KVH, D = 16, 4, 128
THETA = 10000.0
NCORES = 8
TP = 4            # tensor-parallel group size (cores per batch)
QH = H // TP      # q heads per core = 4
QCH = 256         # q columns per attention chunk
NJQ = L // QCH    # 8
NKB = L // 128    # 16 k-blocks / q-blocks
NHC = HID // 128  # 16 hidden-dim chunks
LC = 512          # phase-1 L-chunk width
NLC = L // LC     # 4
BAND_BACK = 3     # k-block lookback; correct while max segment <= 385
SCALE = float(D) ** -0.5
WS = 32.0         # fp8 weight pre-scale (power of two, folded out exactly)
EXP_BIAS = -3.0   # uniform exp bias; cancels in softmax, centers 1/l in fp16


def _band(jq):
    lo = max(0, 2 * jq - BAND_BACK)
    hi = 2 * jq + 1
    return lo, hi


_BANDS = [_band(j) for j in range(NJQ)]
_NBLK = sum(hi - lo + 1 for lo, hi in _BANDS)
_MBASE = np.cumsum([0] + [hi - lo + 1 for lo, hi in _BANDS]).tolist()

_CACHE = {}
LAST_EXEC_NS = None
LAST_RUN_WALL_S = None


def _build_nc():
    import concourse.tile as tile
    from concourse import bacc, mybir

    f32 = mybir.dt.float32
    f16 = mybir.dt.float16
    f8 = mybir.dt.float8e4
    DR = mybir.MatmulPerfMode.DoubleRow
    EXP = mybir.ActivationFunctionType.Exp

    nc = bacc.Bacc(
        "TRN2", target_bir_lowering=False, debug=False, num_devices=NCORES
    )

    xTh = nc.dram_tensor("xTh", [HID, L], f8, kind="ExternalInput").ap()
    xTl = nc.dram_tensor("xTl", [HID, L], f8, kind="ExternalInput").ap()
    wqh = nc.dram_tensor("wqh", [HID, QH * D], f8, kind="ExternalInput").ap()
    wql = nc.dram_tensor("wql", [HID, QH * D], f8, kind="ExternalInput").ap()
    wkh = nc.dram_tensor("wkh", [HID, D], f8, kind="ExternalInput").ap()
    wkl = nc.dram_tensor("wkl", [HID, D], f8, kind="ExternalInput").ap()
    wvh = nc.dram_tensor("wvh", [HID, D], f8, kind="ExternalInput").ap()
    wvl = nc.dram_tensor("wvl", [HID, D], f8, kind="ExternalInput").ap()
    woh = nc.dram_tensor("woh", [QH * D, HID], f8, kind="ExternalInput").ap()
    wol = nc.dram_tensor("wol", [QH * D, HID], f8, kind="ExternalInput").ap()
    cosd = nc.dram_tensor("cosd", [D, L], f16, kind="ExternalInput").ap()
    sind = nc.dram_tensor("sind", [D, L], f16, kind="ExternalInput").ap()
    mskd = nc.dram_tensor("mskd", [_NBLK, 128, QCH], f16, kind="ExternalInput").ap()
    swpd = nc.dram_tensor("swpd", [128, 128], f16, kind="ExternalInput").ap()
    out = nc.dram_tensor("out", [L, HID], f16, kind="ExternalOutput").ap()

    with tile.TileContext(nc) as tc, ExitStack() as top:
        persist = top.enter_context(tc.tile_pool(name="persist", bufs=1))
        kT = persist.tile([128, L], f16, tag="kT", name="kT")
        qT = [
            persist.tile([128, L], f16, tag=f"qT{h}", name=f"qT{h}")
            for h in range(QH)
        ]
        # o in fp8 hi/lo pairs, head-major layout for DoubleRow O-proj pairs
        o8h = persist.tile([128, QH, L], f8, tag="o8h", name="o8h")
        o8l = persist.tile([128, QH, L], f8, tag="o8l", name="o8l")
        vA = persist.tile([128, NKB, 128], f16, tag="vA", name="vA")
        cosT = persist.tile([128, L], f16, tag="cosT", name="cosT")
        sinT = persist.tile([128, L], f16, tag="sinT", name="sinT")
        ones = persist.tile([128, 1], f16, tag="ones", name="ones")
        swp = persist.tile([128, 128], f16, tag="swp", name="swp")
        bias_t = persist.tile([128, 1], f32, tag="biast", name="bias_t")

        nc.vector.memset(ones[:], WS)
        nc.vector.memset(bias_t[:], EXP_BIAS)

        # weights / inputs (live whole kernel)
        wpool = top.enter_context(tc.tile_pool(name="wpool", bufs=1))
        wq_s = [wpool.tile([128, NHC, QH * D], f8, tag=f"wq{i}", name=f"wq_s{i}")
                for i in range(2)]
        wk_s = [wpool.tile([128, NHC, D], f8, tag=f"wk{i}", name=f"wk_s{i}")
                for i in range(2)]
        wv_s = [wpool.tile([128, NHC, D], f8, tag=f"wv{i}", name=f"wv_s{i}")
                for i in range(2)]
        wo_s = [wpool.tile([128, QH, HID], f8, tag=f"wo{i}", name=f"wo_s{i}")
                for i in range(2)]
        xpool = top.enter_context(tc.tile_pool(name="xpool", bufs=2))

        # ---- startup DMAs (single SP queue mostly; HWDGE is one shared
        # ~625ns/issue resource, so batch into few, need-ordered DMAs) ----
        xt0 = [xpool.tile([128, NHC, LC], f8, tag=f"x{i}", name=f"xt0_{i}")
               for i in range(2)]
        nc.sync.dma_start(wk_s[0][:], wkh.rearrange("(c p) d -> p c d", p=128))
        nc.sync.dma_start(wk_s[1][:], wkl.rearrange("(c p) d -> p c d", p=128))
        nc.sync.dma_start(
            xt0[0][:, 0:8, :],
            xTh[0 : 8 * 128, 0:LC].rearrange("(c p) n -> p c n", p=128),
        )
        nc.scalar.dma_start(
            xt0[1][:, 0:8, :],
            xTl[0 : 8 * 128, 0:LC].rearrange("(c p) n -> p c n", p=128),
        )
        nc.sync.dma_start(
            xt0[0][:, 8:16, :],
            xTh[8 * 128 :, 0:LC].rearrange("(c p) n -> p c n", p=128),
        )
        nc.scalar.dma_start(
            xt0[1][:, 8:16, :],
            xTl[8 * 128 :, 0:LC].rearrange("(c p) n -> p c n", p=128),
        )
        nc.scalar.dma_start(cosT[:], cosd[:])
        nc.scalar.dma_start(sinT[:], sind[:])
        nc.scalar.dma_start(swp[:], swpd[:])
        nc.sync.dma_start(wv_s[0][:], wvh.rearrange("(c p) d -> p c d", p=128))
        nc.sync.dma_start(wv_s[1][:], wvl.rearrange("(c p) d -> p c d", p=128))
        nc.sync.dma_start(wq_s[0][:], wqh.rearrange("(c p) d -> p c d", p=128))
        nc.sync.dma_start(wq_s[1][:], wql.rearrange("(c p) d -> p c d", p=128))
        nc.scalar.dma_start(wo_s[0][:], woh.rearrange("(h p) n -> p h n", p=128))
        nc.scalar.dma_start(wo_s[1][:], wol.rearrange("(h p) n -> p h n", p=128))

        tpool = top.enter_context(tc.tile_pool(name="tpool", bufs=3))
        mpool = top.enter_context(tc.tile_pool(name="mpool", bufs=2))
        ppool = top.enter_context(tc.tile_pool(name="ppool", bufs=3))
        spool = top.enter_context(tc.tile_pool(name="spool", bufs=2))
        opool = top.enter_context(tc.tile_pool(name="opool", bufs=2))
        # PSUM: 8 banks. psB [128,512] x2; psS [128,2,256] x2;
        # psO [128,4,256] x1 (2 banks); psL [128,4,256] x1 (2 banks).
        psB = top.enter_context(tc.tile_pool(name="psB", bufs=2, space="PSUM"))
        psS = top.enter_context(tc.tile_pool(name="psS", bufs=2, space="PSUM"))
        psO = top.enter_context(tc.tile_pool(name="psO", bufs=1, space="PSUM"))
        psL = top.enter_context(tc.tile_pool(name="psL", bufs=1, space="PSUM"))

        NCH = NHC // 2  # 8 DoubleRow K-steps (256-contraction each)

        def split3(xt):
            # (x_tile_idx, w_tile_idx) term order for the 3-term hi/lo split
            return ((0, 0), (1, 0), (0, 1))

        def proj_chain(ps, w_tiles, lhs_col0, xt, n0, nn):
            # one 3-term DoubleRow accumulation chain: out [128, nn]
            first, last = (0, 0), (0, 1)
            for xi, wi in split3(xt):
                for c in range(NCH):
                    nc.tensor.matmul(
                        ps[:, n0 : n0 + nn],
                        w_tiles[wi][:, 2 * c : 2 * c + 2,
                                    lhs_col0 : lhs_col0 + 128],
                        xt[xi][:, 2 * c : 2 * c + 2, n0 : n0 + nn],
                        start=((xi, wi) == first and c == 0),
                        stop=((xi, wi) == last and c == NCH - 1),
                        perf_mode=DR,
                    )

        def rope(ps, dst, cols):
            # dst[:, cols] = plain*cos + rotate_half(plain)*sin
            plain = tpool.tile([128, LC], f16, tag="plain", name=f"pl{rope.i}")
            nc.scalar.copy(plain[:], ps[:])
            sw = psB.tile([128, LC], f32, tag="big", name=f"sw{rope.i}")
            nc.tensor.matmul(sw[:], swp[:], plain[:], start=True, stop=True)
            t1 = tpool.tile([128, LC], f16, tag="t1", name=f"t1_{rope.i}")
            nc.vector.tensor_mul(t1[:], plain[:], cosT[:, cols])
            t2 = tpool.tile([128, LC], f16, tag="t2", name=f"t2_{rope.i}")
            nc.vector.tensor_mul(t2[:], sw[:], sinT[:, cols])
            nc.vector.tensor_add(dst[:, cols], t1[:], t2[:])
            rope.i += 1
        rope.i = 0

        def emit_proj(lc):
            cols = slice(lc * LC, (lc + 1) * LC)
            if lc == 0:
                xt = xt0
            else:
                xt = [xpool.tile([128, NHC, LC], f8, tag=f"x{i}",
                                 name=f"xt{lc}_{i}") for i in range(2)]
                nc.sync.dma_start(
                    xt[0][:], xTh[:, cols].rearrange("(c p) n -> p c n", p=128)
                )
                nc.scalar.dma_start(
                    xt[1][:], xTl[:, cols].rearrange("(c p) n -> p c n", p=128)
                )
            # K first: weights arrive earliest
            ps = psB.tile([128, LC], f32, tag="big", name=f"psk{lc}")
            for n0 in range(0, LC, 256):
                proj_chain(ps, wk_s, 0, xt, n0, 256)
            rope(ps, kT, cols)
            # V: directly transposed, 4 k-block column slots in one bank
            vps = psB.tile([128, LC], f32, tag="big", name=f"psv{lc}")
            for kbl in range(4):
                kb = lc * 4 + kbl
                first, last = (0, 0), (0, 1)
                for xi, wi in split3(xt):
                    for c in range(NCH):
                        nc.tensor.matmul(
                            vps[:, kbl * 128 : kbl * 128 + 128],
                            xt[xi][:, 2 * c : 2 * c + 2,
                                   kbl * 128 : kbl * 128 + 128],
                            wv_s[wi][:, 2 * c : 2 * c + 2, :],
                            start=((xi, wi) == first and c == 0),
                            stop=((xi, wi) == last and c == NCH - 1),
                            perf_mode=DR,
                        )
                nc.scalar.copy(
                    vA[:, kb, :], vps[:, kbl * 128 : kbl * 128 + 128]
                )
            # Q heads
            for hb in range(QH):
                ps = psB.tile([128, LC], f32, tag="big", name=f"psq{lc}_{hb}")
                for n0 in range(0, LC, 256):
                    proj_chain(ps, wq_s, hb * 128, xt, n0, 256)
                rope(ps, qT[hb], cols)

        def emit_attn(jq):
            lo, hi = _BANDS[jq]
            nkb = hi - lo + 1
            qs = slice(jq * QCH, (jq + 1) * QCH)
            msk = mpool.tile([128, nkb, QCH], f16, tag="m", name=f"msk{jq}")
            nc.sync.dma_start(
                msk[:],
                mskd[_MBASE[jq] : _MBASE[jq] + nkb].rearrange("k p n -> p k n"),
            )
            # one denominator column slot per head, all at partition 0
            l_ps = psL.tile([128, QH, QCH], f32, tag="l", name=f"l{jq}")
            o_ps = psO.tile([128, QH, QCH], f32, tag="o", name=f"o{jq}")
            for h in range(QH):
                P = ppool.tile([128, nkb, QCH], f16, tag="P", name=f"p{jq}_{h}")
                SB = 2
                for p0 in range(0, nkb, SB):
                    pn = min(SB, nkb - p0)
                    s_ps = psS.tile(
                        [128, SB, QCH], f32, tag="S", name=f"s{jq}_{h}_{p0}"
                    )
                    for i in range(p0, p0 + pn):
                        kb = lo + i
                        nc.tensor.matmul(
                            s_ps[:, i - p0, :],
                            kT[:, kb * 128 : (kb + 1) * 128],
                            qT[h][:, qs],
                            start=True,
                            stop=True,
                        )
                    # exp in fp32 PSUM -> fp16 SBUF; x32-weight scale folded
                    # into the activation scale; then {0,1} mask on DVE 4x
                    nc.scalar.activation(
                        P[:, p0 : p0 + pn, :],
                        s_ps[:, :pn, :],
                        EXP,
                        scale=SCALE / (WS * WS),
                        bias=bias_t[:],
                    )
                    nc.vector.tensor_mul(
                        P[:, p0 : p0 + pn, :],
                        P[:, p0 : p0 + pn, :],
                        msk[:, p0 : p0 + pn, :],
                    )
                # denominator: 32-valued ones folds the 32x of vA out of 1/l
                for i in range(nkb):
                    nc.tensor.matmul(
                        l_ps[0:1, h, :],
                        ones[:],
                        P[:, i, :],
                        start=(i == 0),
                        stop=(i == nkb - 1),
                    )
                # AV (vA holds 32v)
                for i in range(nkb):
                    nc.tensor.matmul(
                        o_ps[:, h, :],
                        vA[:, lo + i, :],
                        P[:, i, :],
                        start=(i == 0),
                        stop=(i == nkb - 1),
                    )
            rc = spool.tile([1, QH, QCH], f16, tag="rc", name=f"rc{jq}")
            with nc.allow_low_precision(
                reason="fp16 1/l scales fp16 outputs; 5e-4 rel ok"
            ):
                nc.vector.reciprocal(rc[:], l_ps[0:1, :, :])
            for h in range(QH):
                r_bc = spool.tile(
                    [128, QCH], f16, tag="lbc", bufs=4, name=f"lb{jq}_{h}"
                )
                nc.gpsimd.partition_broadcast(r_bc[:], rc[0:1, h, :])
                t16 = tpool.tile([128, QCH], f16, tag="t16", name=f"t16_{jq}_{h}")
                nc.vector.tensor_mul(t16[:], o_ps[:, h, :], r_bc[:])
                nc.scalar.copy(o8h[:, h, qs], t16[:])
                nc.gpsimd.tensor_sub(o8l[:, h, qs], t16[:], o8h[:, h, qs])

        def emit_oproj(qb):
            ob = opool.tile([128, HID], f16, tag="ob", name=f"ob{qb}")
            qsl = slice(qb * 128, (qb + 1) * 128)
            for hc in range(HID // 512):
                f_ps = psB.tile([128, 512], f32, tag="big", name=f"f{qb}_{hc}")
                for n0 in range(0, 512, 256):
                    col = slice(hc * 512 + n0, hc * 512 + n0 + 256)
                    terms = ((o8h, 0), (o8l, 0), (o8h, 1))
                    for ti, (ot, wi) in enumerate(terms):
                        for hh in range(2):
                            nc.tensor.matmul(
                                f_ps[:, n0 : n0 + 256],
                                ot[:, 2 * hh : 2 * hh + 2, qsl],
                                wo_s[wi][:, 2 * hh : 2 * hh + 2, col],
                                start=(ti == 0 and hh == 0),
                                stop=(ti == 2 and hh == 1),
                                perf_mode=DR,
                            )
                # PSUM->SBUF copies split across Act/DVE to balance load
                if hc % 2 == 0:
                    nc.scalar.copy(ob[:, hc * 512 : (hc + 1) * 512], f_ps[:])
                else:
                    nc.vector.tensor_copy(ob[:, hc * 512 : (hc + 1) * 512], f_ps[:])
            nc.sync.dma_start(out[qb * 128 : (qb + 1) * 128, :], ob[:])

        for lc in range(NLC):
            emit_proj(lc)
            emit_attn(2 * lc)
            emit_attn(2 * lc + 1)
            for qb in range(4 * lc, 4 * lc + 4):
                emit_oproj(qb)

    nc.compile()
    return nc


def _get_nc():
    if "nc" not in _CACHE:
        _CACHE["nc"] = _build_nc()
    return _CACHE["nc"]


def kernel(hidden_states, Wq, Wk, Wv, Wo, sid, position_ids):
    global LAST_EXEC_NS, LAST_RUN_WALL_S
    import time

    import ml_dtypes
    from concourse.bass_utils import run_bass_kernel_spmd

    f8 = ml_dtypes.float8_e4m3
    f16 = np.float16

    hidden = np.asarray(hidden_states, dtype=np.float32)
    Wq = np.asarray(Wq, dtype=np.float32)
    Wk = np.asarray(Wk, dtype=np.float32)
    Wv = np.asarray(Wv, dtype=np.float32)
    Wo = np.asarray(Wo, dtype=np.float32)
    sid = np.asarray(sid)
    position_ids = np.asarray(position_ids)

    nc = _get_nc()

    def split8(a):
        h = a.astype(f8)
        l = (a - h.astype(np.float32)).astype(f8)
        return np.ascontiguousarray(h), np.ascontiguousarray(l)

    swpm = np.zeros((128, 128), f16)
    swpm[(np.arange(128) + 64) % 128, np.arange(128)] = 1.0

    in_maps = []
    perms = []
    for b in range(B):
        s = sid[b].astype(np.int64)
        perm = np.argsort(s, kind="stable")
        perms.append(perm)
        st = s[perm]
        seg_max = int(np.bincount(st, minlength=1).max())
        assert seg_max <= BAND_BACK * 128 + 1, (
            f"segment length {seg_max} exceeds supported band"
        )

        pos = position_ids[b][perm].astype(np.float32)
        inv = (
            1.0
            / (THETA ** (np.arange(0, D, 2, dtype=np.float32) / np.float32(D)))
        ).astype(np.float32)
        fr = pos[:, None] * inv[None, :]
        emb = np.concatenate([fr, fr], axis=1)  # [L, D]
        cosT = np.ascontiguousarray(np.cos(emb).T.astype(f16))
        sinT = np.sin(emb).T.astype(np.float32).copy()
        sinT[: D // 2] *= -1.0  # fold rotate_half sign
        sinT = np.ascontiguousarray(sinT.astype(f16))

        xT = hidden[b].T[:, perm]
        xh, xl = split8(xT)

        msk = np.zeros((_NBLK, 128, QCH), f16)
        ki = np.arange(128)
        qi = np.arange(QCH)
        for jq in range(NJQ):
            lo, hi = _BANDS[jq]
            for i in range(hi - lo + 1):
                kb = lo + i
                kidx = kb * 128 + ki
                qidx = jq * QCH + qi
                m = (st[kidx][:, None] == st[qidx][None, :]) & (
                    kidx[:, None] <= qidx[None, :]
                )
                msk[_MBASE[jq] + i] = m.astype(f16)

        for g in range(TP):
            wqh, wql = split8(Wq[g * 512 : (g + 1) * 512].T * WS)
            wkh, wkl = split8(Wk[g * 128 : (g + 1) * 128].T * WS)
            wvh, wvl = split8(Wv[g * 128 : (g + 1) * 128].T * WS)
            woh, wol = split8(Wo[:, g * 512 : (g + 1) * 512].T * WS)
            in_maps.append(
                dict(
                    xTh=xh, xTl=xl,
                    wqh=wqh, wql=wql,
                    wkh=wkh, wkl=wkl,
                    wvh=wvh, wvl=wvl,
                    woh=woh, wol=wol,
                    cosd=cosT,
                    sind=sinT,
                    mskd=msk,
                    swpd=swpm,
                )
            )

    t0 = time.time()
    res = run_bass_kernel_spmd(nc, in_maps, core_ids=list(range(NCORES)))
    LAST_RUN_WALL_S = time.time() - t0
    LAST_EXEC_NS = res.exec_time_ns

    full = np.empty((B, L, HID), np.float32)
    for b in range(B):
        acc = np.asarray(res.results[4 * b]["out"]).astype(np.float32)
        for g in range(1, TP):
            acc += np.asarray(res.results[4 * b + g]["out"]).astype(np.float32)
        acc /= np.float32(WS)
        unp = np.empty_like(acc)
        unp[perms[b]] = acc
        full[b] = unp
    return full
